# revision 18
# baseline (speedup 1.0000x reference)
"""Trainium2 Bass kernel for nn_BackBone (LSTM backbone + fc + outer-product head).

Data-parallel over batch across 8 NeuronCores. Per core (b_loc=1024 rows):
  - history is pre-cast to fp16 and pre-transposed on the host to
    xT[d, t, j, b] so the PE never runs transposes and the HBM read is half
    the fp32 size; loads stream in t-blocks (earliest timesteps first) on
    the sync HWDGE queue; small tensors (weights, cn, pref) load on the
    scalar HWDGE queue so their completion isn't serialized behind the bulk
  - all matmuls fp16xfp16, PSUM fp32, gate bias folded into ScalarE's
    per-partition bias port
  - full-width recurrence: each gate accumulates into one [128,1024] PSUM
    tile (2 banks); per step 24 input-projection matmuls run first, the 8
    W_hh matmuls last, so the serial h->gates latency (~3us of ACT+DVE) is
    hidden under the next step's input projection (~6.5us)
  - head split: y2 = relu(cn_t.T @ fc_w.T + fc_b) half of the einsum runs
    during the recurrence on DVE (l-quad chunks, each shorter than the
    per-step DVE slack); the h-half runs in the tail, stores streaming
    behind the final hidden state
  - einsum on DVE writes fp32 directly; all DMA is HWDGE (no casts)
"""
import numpy as np

import concourse.bacc as bacc
import concourse.mybir as mybir
import concourse.tile as tile
from concourse import bass_utils

F32 = mybir.dt.float32
F16 = mybir.dt.float16
AF = mybir.ActivationFunctionType

T = 20
D = 340
H = 128
G4 = 4
E = 32
L = 10
M3 = 3
DCH = [(0, 128), (128, 256), (256, 340)]
TBLK = [(0, 1), (1, 2), (2, 4), (4, 8), (8, 12), (12, 16), (16, 20)]
N_CORES = 8


def build_program(b_loc: int):
    BG = b_loc                    # single group
    assert BG % 512 == 0
    NJ = BG // 128
    NH2 = BG // 512               # 512-wide halves per matmul dest bank

    nc = bacc.Bacc("TRN2", target_bir_lowering=False, debug=False)
    hist = nc.dram_tensor("hist_t", (D, T, NJ, 128), F16, kind="ExternalInput").ap()
    cnt = nc.dram_tensor("cn_t", (E, b_loc), F16, kind="ExternalInput").ap()
    pref = nc.dram_tensor("pref16", (128, b_loc // 128, L, M3), F16,
                          kind="ExternalInput").ap()
    wih = nc.dram_tensor("w_ih_t", (D, 4 * H), F16, kind="ExternalInput").ap()
    whh = nc.dram_tensor("w_hh_t", (H, 4 * H), F16, kind="ExternalInput").ap()
    bias4 = nc.dram_tensor("bias4", (H, G4), F32, kind="ExternalInput").ap()
    fcw = nc.dram_tensor("fc_w_t", (E, H), F16, kind="ExternalInput").ap()
    fcb = nc.dram_tensor("fc_b_row", (1, H), F16, kind="ExternalInput").ap()
    ones1 = nc.dram_tensor("ones_row", (1, 128), F16, kind="ExternalInput").ap()
    ident = nc.dram_tensor("ident", (128, 128), F16, kind="ExternalInput").ap()
    out = nc.dram_tensor("out", (b_loc, L * 256 * M3), F32, kind="ExternalOutput").ap()

    with tile.TileContext(nc) as tc:
        with tc.tile_pool(name="wpool", bufs=1) as wpool, \
             tc.tile_pool(name="main", bufs=1) as pool, \
             tc.tile_pool(name="psum", bufs=1, space="PSUM") as pspool:

            # ---- persistent fp16 xT tiles; the t=0 block goes out first
            # on the scalar HWDGE queue while W_ih issues on sync, so the
            # first gate matmul can start ~10us in ----
            xt_tiles = [
                pool.tile([c1 - c0, T, NJ, 128], F16, name=f"xt{k}", tag=f"xt{k}")
                for k, (c0, c1) in enumerate(DCH)
            ]
            t0, t1 = TBLK[0]
            for k, (c0, c1) in enumerate(DCH):
                nc.scalar.dma_start(
                    xt_tiles[k][:, t0:t1, :, :], hist[c0:c1, t0:t1, :, :])
            wih_t = []
            for k, (c0, c1) in enumerate(DCH):
                wt_ = wpool.tile([c1 - c0, 4 * H], F16, name=f"wih{k}")
                nc.sync.dma_start(wt_[:], wih[c0:c1, :])
                wih_t.append(wt_)
            whh_t = wpool.tile([H, 4 * H], F16, name="whh_t")
            nc.sync.dma_start(whh_t[:], whh)
            bias_t = wpool.tile([H, G4], F32, name="bias_t")
            nc.sync.dma_start(bias_t[:], bias4)
            for t0, t1 in TBLK[1:]:
                for k, (c0, c1) in enumerate(DCH):
                    nc.sync.dma_start(
                        xt_tiles[k][:, t0:t1, :, :], hist[c0:c1, t0:t1, :, :])
            ident_t = wpool.tile([128, 128], F16, name="ident_t")
            nc.sync.dma_start(ident_t[:], ident)
            pf_all = wpool.tile([128, NJ, L, M3], F16, name="pf_all")
            nc.sync.dma_start(pf_all[:], pref)
            cnt_t = wpool.tile([E, b_loc], F16, name="cnt_t")
            nc.scalar.dma_start(cnt_t[:], cnt)
            fcw_t = wpool.tile([E, H], F16, name="fcw_t")
            nc.scalar.dma_start(fcw_t[:], fcw)
            fcb_t = wpool.tile([1, H], F16, name="fcb_t")
            nc.scalar.dma_start(fcb_t[:], fcb)
            ones_t = wpool.tile([1, 128], F16, name="ones_t")
            nc.scalar.dma_start(ones_t[:], ones1)

            def emit_einsum_half(j, y3, n_off):
                """Einsum for a 128-wide n-half given replicated y3 =
                y x ones_3 (dense inner operand): one fp16 DVE multiply and
                one SWDGE fp16->fp32 cast store per half."""
                rows = j * 128
                out3 = out[rows:rows + 128, :].rearrange(
                    "p (l nm) -> p l nm", l=L)
                for l0 in range(0, L, 5):
                    ol = pool.tile([128, 5, 128, M3], F16, name="ol",
                                   tag="outl", bufs=6)
                    y_b = y3[:, None, :, :].broadcast_to([128, 5, 128, M3])
                    p_b = pf_all[:, j, l0:l0 + 5, None, :].broadcast_to(
                        [128, 5, 128, M3])
                    nc.vector.tensor_mul(ol[:], y_b, p_b)
                    nc.gpsimd.dma_start(
                        out3[:, l0:l0 + 5, n_off * 3:n_off * 3 + 384], ol[:])

            def emit_y2_head(j):
                y2p = pspool.tile([128, 128], F32, name="y2p",
                                  tag="gp", bufs=8)
                nc.tensor.matmul(y2p[:], cnt_t[:, j * 128:(j + 1) * 128],
                                 fcw_t[:], start=True, stop=False)
                nc.tensor.matmul(y2p[:], ones_t[:], fcb_t[:],
                                 start=False, stop=True)
                y3 = pool.tile([128, 128, M3], F16, name="y3",
                               tag="y3", bufs=4)
                nc.scalar.activation(
                    y3[:], y2p[:, :, None].broadcast_to([128, 128, M3]),
                    AF.Relu)
                emit_einsum_half(j, y3, 128)

            def emit_h_head_j(h_final, ch, jloc):
                # h_final covers batch cols [ch*512, ch*512+512); jloc in 0..3
                j = ch * 4 + jloc
                tp_h = pspool.tile([128, 128], F16, name="tp_h",
                                   tag="gp", bufs=8)
                nc.tensor.matmul(
                    tp_h[:],
                    h_final[:, jloc * 128:(jloc + 1) * 128],
                    ident_t[:], is_transpose=True, start=True, stop=True)
                y3 = pool.tile([128, 128, M3], F16, name="y3",
                               tag="y3", bufs=4)
                nc.scalar.copy(
                    y3[:], tp_h[:, :, None].broadcast_to([128, 128, M3]))
                emit_einsum_half(j, y3, 0)

            # ---- fused projection + LSTM recurrence, two phase-lagged
            # half-width chains: chain 1 runs LAG steps behind chain 0, so
            # chain 0's half of the h-einsum output streams to HBM while
            # chain 1 finishes, halving the tail store volume ----
            LAG = 3
            h_prev = [None, None]
            c_prev = [None, None]

            def half_step(ch, t):
                gates = [pool.tile([128, 512], F16, name=f"gate{g}",
                                   tag=f"g{g}c{ch}", bufs=2)
                         for g in range(G4)]
                c_t = pool.tile([128, 512], F16, name="c_t",
                                tag=f"c{ch}", bufs=2)
                tc_t = pool.tile([128, 512], F16, name="tc_t",
                                 tag=f"tc{ch}", bufs=2)
                h_t = pool.tile([128, 512], F16, name="h_t",
                                tag=f"h{ch}", bufs=2)
                gps = [pspool.tile([128, 512], F32, name="gp",
                                   tag="gp", bufs=8) for g in range(G4)]
                for k in range(3):
                    for g in range(G4):
                        gsl = slice(g * 128, (g + 1) * 128)
                        nc.tensor.matmul(
                            gps[g][:], wih_t[k][:, gsl],
                            xt_tiles[k][:, t, ch * 4:(ch + 1) * 4, :],
                            start=(k == 0),
                            stop=(k == 2 and t == 0))
                if t > 0:
                    for g in range(G4):
                        gsl = slice(g * 128, (g + 1) * 128)
                        nc.tensor.matmul(gps[g][:], whh_t[:, gsl],
                                         h_prev[ch][:],
                                         start=False, stop=True)
                for g in range(G4):
                    func = AF.Tanh if g == 2 else AF.Sigmoid
                    nc.scalar.activation(gates[g][:], gps[g][:], func,
                                         bias=bias_t[:, g:g + 1], scale=1.0)
                i_t, f_t, g_t, o_t = gates
                if t == 0:
                    nc.vector.tensor_mul(c_t[:], i_t[:], g_t[:])
                else:
                    t1 = pool.tile([128, 512], F16, name="t1",
                                   tag=f"t1c{ch}", bufs=2)
                    nc.vector.tensor_mul(t1[:], f_t[:], c_prev[ch][:])
                    t2 = pool.tile([128, 512], F16, name="t2",
                                   tag=f"t2c{ch}", bufs=2)
                    nc.vector.tensor_mul(t2[:], i_t[:], g_t[:])
                    nc.vector.tensor_add(c_t[:], t1[:], t2[:])
                nc.scalar.activation(tc_t[:], c_t[:], AF.Tanh)
                nc.vector.tensor_mul(h_t[:], o_t[:], tc_t[:])
                h_prev[ch], c_prev[ch] = h_t, c_t
                # y2 head mid-stream: its stores land in the DMA lull
                # after the xt loads finish, not on top of them
                if ch == 0 and 8 <= t <= 11:
                    emit_y2_head(2 * (t - 8))
                    emit_y2_head(2 * (t - 8) + 1)

            sched = [(0, t) for t in range(LAG)]
            for t in range(LAG, T):
                sched.append((1, t - LAG))
                sched.append((0, t))
            sched += [(1, t) for t in range(T - LAG, T)]
            # chain 0's h-head interleaves into chain 1's trailing solo
            # steps (emitting it inside chain 0's last step would stall
            # chain 1's matmuls behind the h_0-latency-bound transposes)
            for ch, t in sched:
                half_step(ch, t)
                if ch == 1 and t in (T - LAG, T - LAG + 1):
                    jb = 2 * (t - (T - LAG))
                    emit_h_head_j(h_prev[0], 0, jb)
                    emit_h_head_j(h_prev[0], 0, jb + 1)
            for jloc in range(4):
                emit_h_head_j(h_prev[1], 1, jloc)

    nc.compile()
    return nc


def prep_in_maps(inputs, n_cores: int, b_loc: int):
    history = np.asarray(inputs["history"], np.float32)
    cluster = np.asarray(inputs["cluster_num"], np.float32)
    pref = np.asarray(inputs["pref"], np.float32)
    w_ih = np.asarray(inputs["W_ih"], np.float32)
    w_hh = np.asarray(inputs["W_hh"], np.float32)
    b_ih = np.asarray(inputs["b_ih"], np.float32)
    b_hh = np.asarray(inputs["b_hh"], np.float32)
    fc_w = np.asarray(inputs["fc_w"], np.float32)
    fc_b = np.asarray(inputs["fc_b"], np.float32)

    hist16 = history.reshape(-1, T, D).astype(np.float16)
    NJ = b_loc // 128

    shared = {
        "w_ih_t": np.ascontiguousarray(w_ih.T.astype(np.float16)),
        "w_hh_t": np.ascontiguousarray(w_hh.T.astype(np.float16)),
        "bias4": np.ascontiguousarray((b_ih + b_hh).reshape(G4, H).T),  # [128,4]
        "fc_w_t": np.ascontiguousarray(fc_w.T.astype(np.float16)),
        "fc_b_row": np.ascontiguousarray(fc_b.reshape(1, H).astype(np.float16)),
        "ones_row": np.ones((1, 128), np.float16),
        "ident": np.eye(128, dtype=np.float16),
    }
    in_maps = []
    for c in range(n_cores):
        r0, r1 = c * b_loc, (c + 1) * b_loc
        # [b_loc, T, D] -> [D, T, b_loc] -> [D, T, NJ, 128]
        ht = np.ascontiguousarray(hist16[r0:r1].transpose(2, 1, 0))
        in_maps.append({
            "hist_t": ht.reshape(D, T, NJ, 128),
            "cn_t": np.ascontiguousarray(cluster[r0:r1].T.astype(np.float16)),
            # [b_loc, L, M3] -> [128, NJ, L, M3] so the pf DMA is one
            # contiguous block per partition
            "pref16": np.ascontiguousarray(
                pref[r0:r1].reshape(NJ, 128, L, M3).astype(np.float16)
                .transpose(1, 0, 2, 3)),
            **shared,
        })
    return in_maps


def run(inputs, n_cores: int = N_CORES, trace: bool = False):
    B = np.asarray(inputs["history"]).shape[0]
    b_loc = B // n_cores
    nc = build_program(b_loc)
    in_maps = prep_in_maps(inputs, n_cores, b_loc)
    res = bass_utils.run_bass_kernel_spmd(
        nc, in_maps, core_ids=list(range(n_cores)), trace=trace)
    outs = [res.results[c]["out"].reshape(b_loc, L, 256 * M3)
            for c in range(n_cores)]
    return np.concatenate(outs, axis=0), res


def kernel(**inputs) -> np.ndarray:
    out, _ = run(inputs, N_CORES)
    return out


# revision 19
# speedup vs baseline: 1.0552x; 1.0552x over previous
"""Trainium2 Bass kernel for nn_BackBone (LSTM backbone + fc + outer-product head).

Data-parallel over batch across 8 NeuronCores. Per core (b_loc=1024 rows):
  - history is pre-cast to fp16 and pre-transposed on the host to
    xT[d, t, j, b] so the PE never runs transposes and the HBM read is half
    the fp32 size; loads stream in t-blocks (earliest timesteps first) on
    the sync HWDGE queue; small tensors (weights, cn, pref) load on the
    scalar HWDGE queue so their completion isn't serialized behind the bulk
  - all matmuls fp16xfp16, PSUM fp32, gate bias folded into ScalarE's
    per-partition bias port
  - full-width recurrence: each gate accumulates into one [128,1024] PSUM
    tile (2 banks); per step 24 input-projection matmuls run first, the 8
    W_hh matmuls last, so the serial h->gates latency (~3us of ACT+DVE) is
    hidden under the next step's input projection (~6.5us)
  - head split: y2 = relu(cn_t.T @ fc_w.T + fc_b) half of the einsum runs
    during the recurrence on DVE (l-quad chunks, each shorter than the
    per-step DVE slack); the h-half runs in the tail, stores streaming
    behind the final hidden state
  - einsum on DVE writes fp32 directly; all DMA is HWDGE (no casts)
"""
import numpy as np

import concourse.bacc as bacc
import concourse.mybir as mybir
import concourse.tile as tile
from concourse import bass_utils

F32 = mybir.dt.float32
F16 = mybir.dt.float16
AF = mybir.ActivationFunctionType

T = 20
D = 340
H = 128
G4 = 4
E = 32
L = 10
M3 = 3
DCH = [(0, 128), (128, 256), (256, 340)]
TBLK = [(0, 1), (1, 2), (2, 4), (4, 8), (8, 12), (12, 16), (16, 20)]
N_CORES = 8


def build_program(b_loc: int):
    BG = b_loc                    # single group
    assert BG % 512 == 0
    NJ = BG // 128
    NH2 = BG // 512               # 512-wide halves per matmul dest bank

    nc = bacc.Bacc("TRN2", target_bir_lowering=False, debug=False)
    hist = nc.dram_tensor("hist_t", (D, T, NJ, 128), F16, kind="ExternalInput").ap()
    cnt = nc.dram_tensor("cn_t", (E, b_loc), F16, kind="ExternalInput").ap()
    pref = nc.dram_tensor("pref16", (128, b_loc // 128, L, M3), F16,
                          kind="ExternalInput").ap()
    wih = nc.dram_tensor("w_ih_t", (D, 4 * H), F16, kind="ExternalInput").ap()
    whh = nc.dram_tensor("w_hh_t", (H, 4 * H), F16, kind="ExternalInput").ap()
    bias4 = nc.dram_tensor("bias4", (H, G4), F32, kind="ExternalInput").ap()
    fcw = nc.dram_tensor("fc_w_t", (E, H), F16, kind="ExternalInput").ap()
    fcb = nc.dram_tensor("fc_b_row", (1, H), F16, kind="ExternalInput").ap()
    ones1 = nc.dram_tensor("ones_row", (1, 128), F16, kind="ExternalInput").ap()
    ident = nc.dram_tensor("ident", (128, 128), F16, kind="ExternalInput").ap()
    out = nc.dram_tensor("out", (b_loc, L * 256 * M3), F32, kind="ExternalOutput").ap()

    with tile.TileContext(nc) as tc:
        with tc.tile_pool(name="wpool", bufs=1) as wpool, \
             tc.tile_pool(name="main", bufs=1) as pool, \
             tc.tile_pool(name="psum", bufs=1, space="PSUM") as pspool:

            # ---- persistent fp16 xT tiles; the t=0 block goes out first
            # on the scalar HWDGE queue while W_ih issues on sync, so the
            # first gate matmul can start ~10us in ----
            xt_tiles = [
                pool.tile([c1 - c0, T, NJ, 128], F16, name=f"xt{k}", tag=f"xt{k}")
                for k, (c0, c1) in enumerate(DCH)
            ]
            t0, t1 = TBLK[0]
            for k, (c0, c1) in enumerate(DCH):
                nc.scalar.dma_start(
                    xt_tiles[k][:, t0:t1, :, :], hist[c0:c1, t0:t1, :, :])
            wih_t = []
            for k, (c0, c1) in enumerate(DCH):
                wt_ = wpool.tile([c1 - c0, 4 * H], F16, name=f"wih{k}")
                nc.sync.dma_start(wt_[:], wih[c0:c1, :])
                wih_t.append(wt_)
            whh_t = wpool.tile([H, 4 * H], F16, name="whh_t")
            nc.sync.dma_start(whh_t[:], whh)
            bias_t = wpool.tile([H, G4], F32, name="bias_t")
            nc.sync.dma_start(bias_t[:], bias4)
            for t0, t1 in TBLK[1:]:
                for k, (c0, c1) in enumerate(DCH):
                    nc.sync.dma_start(
                        xt_tiles[k][:, t0:t1, :, :], hist[c0:c1, t0:t1, :, :])
            ident_t = wpool.tile([128, 128], F16, name="ident_t")
            nc.sync.dma_start(ident_t[:], ident)
            pf_all = wpool.tile([128, NJ, L, M3], F16, name="pf_all")
            nc.sync.dma_start(pf_all[:], pref)
            cnt_t = wpool.tile([E, b_loc], F16, name="cnt_t")
            nc.scalar.dma_start(cnt_t[:], cnt)
            fcw_t = wpool.tile([E, H], F16, name="fcw_t")
            nc.scalar.dma_start(fcw_t[:], fcw)
            fcb_t = wpool.tile([1, H], F16, name="fcb_t")
            nc.scalar.dma_start(fcb_t[:], fcb)
            ones_t = wpool.tile([1, 128], F16, name="ones_t")
            nc.scalar.dma_start(ones_t[:], ones1)

            def emit_einsum_half(j, y3, n_off):
                """Einsum for a 128-wide n-half given replicated y3 =
                y x ones_3 (dense inner operand): one fp16 DVE multiply and
                one SWDGE fp16->fp32 cast store per half."""
                rows = j * 128
                out3 = out[rows:rows + 128, :].rearrange(
                    "p (l nm) -> p l nm", l=L)
                for l0 in range(0, L, 5):
                    ol = pool.tile([128, 5, 128, M3], F16, name="ol",
                                   tag="outl", bufs=6)
                    y_b = y3[:, None, :, :].broadcast_to([128, 5, 128, M3])
                    p_b = pf_all[:, j, l0:l0 + 5, None, :].broadcast_to(
                        [128, 5, 128, M3])
                    nc.vector.tensor_mul(ol[:], y_b, p_b)
                    nc.gpsimd.dma_start(
                        out3[:, l0:l0 + 5, n_off * 3:n_off * 3 + 384], ol[:])

            def emit_y2_head(j):
                y2p = pspool.tile([128, 128], F32, name="y2p",
                                  tag="gp", bufs=8)
                nc.tensor.matmul(y2p[:], cnt_t[:, j * 128:(j + 1) * 128],
                                 fcw_t[:], start=True, stop=False)
                nc.tensor.matmul(y2p[:], ones_t[:], fcb_t[:],
                                 start=False, stop=True)
                y3 = pool.tile([128, 128, M3], F16, name="y3",
                               tag="y3", bufs=4)
                nc.scalar.activation(
                    y3[:], y2p[:, :, None].broadcast_to([128, 128, M3]),
                    AF.Relu)
                emit_einsum_half(j, y3, 128)

            def emit_h_head_j(h_final, ch, jloc):
                # h_final covers batch cols [ch*512, ch*512+512); jloc in 0..3
                j = ch * 4 + jloc
                tp_h = pspool.tile([128, 128], F16, name="tp_h",
                                   tag="gp", bufs=8)
                nc.tensor.matmul(
                    tp_h[:],
                    h_final[:, jloc * 128:(jloc + 1) * 128],
                    ident_t[:], is_transpose=True, start=True, stop=True)
                y3 = pool.tile([128, 128, M3], F16, name="y3",
                               tag="y3", bufs=4)
                nc.scalar.copy(
                    y3[:], tp_h[:, :, None].broadcast_to([128, 128, M3]))
                emit_einsum_half(j, y3, 0)

            # ---- fused projection + LSTM recurrence, two phase-lagged
            # half-width chains: chain 1 runs LAG steps behind chain 0, so
            # chain 0's half of the h-einsum output streams to HBM while
            # chain 1 finishes, halving the tail store volume ----
            LAG = 4
            h_prev = [None, None]
            c_prev = [None, None]

            def half_step(ch, t):
                gates = [pool.tile([128, 512], F16, name=f"gate{g}",
                                   tag=f"g{g}c{ch}", bufs=2)
                         for g in range(G4)]
                c_t = pool.tile([128, 512], F16, name="c_t",
                                tag=f"c{ch}", bufs=2)
                tc_t = pool.tile([128, 512], F16, name="tc_t",
                                 tag=f"tc{ch}", bufs=2)
                h_t = pool.tile([128, 512], F16, name="h_t",
                                tag=f"h{ch}", bufs=2)
                gps = [pspool.tile([128, 512], F32, name="gp",
                                   tag="gp", bufs=8) for g in range(G4)]
                for k in range(3):
                    for g in range(G4):
                        gsl = slice(g * 128, (g + 1) * 128)
                        nc.tensor.matmul(
                            gps[g][:], wih_t[k][:, gsl],
                            xt_tiles[k][:, t, ch * 4:(ch + 1) * 4, :],
                            start=(k == 0),
                            stop=(k == 2 and t == 0))
                if t > 0:
                    for g in range(G4):
                        gsl = slice(g * 128, (g + 1) * 128)
                        nc.tensor.matmul(gps[g][:], whh_t[:, gsl],
                                         h_prev[ch][:],
                                         start=False, stop=True)
                for g in range(G4):
                    func = AF.Tanh if g == 2 else AF.Sigmoid
                    nc.scalar.activation(gates[g][:], gps[g][:], func,
                                         bias=bias_t[:, g:g + 1], scale=1.0)
                i_t, f_t, g_t, o_t = gates
                if t == 0:
                    nc.vector.tensor_mul(c_t[:], i_t[:], g_t[:])
                else:
                    t1 = pool.tile([128, 512], F16, name="t1",
                                   tag=f"t1c{ch}", bufs=2)
                    nc.vector.tensor_mul(t1[:], f_t[:], c_prev[ch][:])
                    t2 = pool.tile([128, 512], F16, name="t2",
                                   tag=f"t2c{ch}", bufs=2)
                    nc.vector.tensor_mul(t2[:], i_t[:], g_t[:])
                    nc.vector.tensor_add(c_t[:], t1[:], t2[:])
                nc.scalar.activation(tc_t[:], c_t[:], AF.Tanh)
                nc.vector.tensor_mul(h_t[:], o_t[:], tc_t[:])
                h_prev[ch], c_prev[ch] = h_t, c_t
                # y2 head mid-stream: its stores land in the DMA lull
                # after the xt loads finish, not on top of them
                if ch == 0 and 8 <= t <= 11:
                    emit_y2_head(2 * (t - 8))
                    emit_y2_head(2 * (t - 8) + 1)

            sched = [(0, t) for t in range(LAG)]
            for t in range(LAG, T):
                sched.append((1, t - LAG))
                sched.append((0, t))
            sched += [(1, t) for t in range(T - LAG, T)]
            # chain 0's h-head interleaves into chain 1's trailing solo
            # steps (emitting it inside chain 0's last step would stall
            # chain 1's matmuls behind the h_0-latency-bound transposes)
            for ch, t in sched:
                half_step(ch, t)
                if ch == 1 and t in (T - LAG, T - LAG + 1):
                    jb = 2 * (t - (T - LAG))
                    emit_h_head_j(h_prev[0], 0, jb)
                    emit_h_head_j(h_prev[0], 0, jb + 1)
            for jloc in range(4):
                emit_h_head_j(h_prev[1], 1, jloc)

    nc.compile()
    return nc


def prep_in_maps(inputs, n_cores: int, b_loc: int):
    history = np.asarray(inputs["history"], np.float32)
    cluster = np.asarray(inputs["cluster_num"], np.float32)
    pref = np.asarray(inputs["pref"], np.float32)
    w_ih = np.asarray(inputs["W_ih"], np.float32)
    w_hh = np.asarray(inputs["W_hh"], np.float32)
    b_ih = np.asarray(inputs["b_ih"], np.float32)
    b_hh = np.asarray(inputs["b_hh"], np.float32)
    fc_w = np.asarray(inputs["fc_w"], np.float32)
    fc_b = np.asarray(inputs["fc_b"], np.float32)

    hist16 = history.reshape(-1, T, D).astype(np.float16)
    NJ = b_loc // 128

    shared = {
        "w_ih_t": np.ascontiguousarray(w_ih.T.astype(np.float16)),
        "w_hh_t": np.ascontiguousarray(w_hh.T.astype(np.float16)),
        "bias4": np.ascontiguousarray((b_ih + b_hh).reshape(G4, H).T),  # [128,4]
        "fc_w_t": np.ascontiguousarray(fc_w.T.astype(np.float16)),
        "fc_b_row": np.ascontiguousarray(fc_b.reshape(1, H).astype(np.float16)),
        "ones_row": np.ones((1, 128), np.float16),
        "ident": np.eye(128, dtype=np.float16),
    }
    in_maps = []
    for c in range(n_cores):
        r0, r1 = c * b_loc, (c + 1) * b_loc
        # [b_loc, T, D] -> [D, T, b_loc] -> [D, T, NJ, 128]
        ht = np.ascontiguousarray(hist16[r0:r1].transpose(2, 1, 0))
        in_maps.append({
            "hist_t": ht.reshape(D, T, NJ, 128),
            "cn_t": np.ascontiguousarray(cluster[r0:r1].T.astype(np.float16)),
            # [b_loc, L, M3] -> [128, NJ, L, M3] so the pf DMA is one
            # contiguous block per partition
            "pref16": np.ascontiguousarray(
                pref[r0:r1].reshape(NJ, 128, L, M3).astype(np.float16)
                .transpose(1, 0, 2, 3)),
            **shared,
        })
    return in_maps


def run(inputs, n_cores: int = N_CORES, trace: bool = False):
    B = np.asarray(inputs["history"]).shape[0]
    b_loc = B // n_cores
    nc = build_program(b_loc)
    in_maps = prep_in_maps(inputs, n_cores, b_loc)
    res = bass_utils.run_bass_kernel_spmd(
        nc, in_maps, core_ids=list(range(n_cores)), trace=trace)
    outs = [res.results[c]["out"].reshape(b_loc, L, 256 * M3)
            for c in range(n_cores)]
    return np.concatenate(outs, axis=0), res


def kernel(**inputs) -> np.ndarray:
    out, _ = run(inputs, N_CORES)
    return out
